# revision 4
# baseline (speedup 1.0000x reference)
"""Kohonen SOM distance-matrix kernel for Trainium2 (Bass/Tile).

Computes sqrt(max(||x||^2 + ||w||^2 - 2 x.w, 0)) for x [32768, 256] against a
codebook w [2500, 256] -> out [32768, 2500], data-parallel over 8 NeuronCores
(batch sharded, codebook replicated).

Device strategy per core (batch shard of 4096 rows):
  - Host preps transposed operands:
      xt [256, 4096] = x.T (float32r for full-rate PE), wt [256, 2500] = (-2 w).T
      xsq [4096] = ||x||^2, wsq [1, 2500] = ||w||^2 (float32)
      (matmul3 mode: extra augmented rows xaug/waug fold the norms into the
      contraction instead)
  - TensorE (float32r: FP22 multiply, FP32 accumulate, full rate for N>=256)
    computes cross = -2 x.w into PSUM tiles [128, 500].
  - bias_add mode: VectorE adds broadcast ||w||^2 in PSUM, ScalarE computes
    sqrt(psum + ||x||^2) via per-partition bias, PSUM -> SBUF, then DMA out.
"""

import os

import numpy as np

N_CORES = 8
BATCH = 32768
BS = BATCH // N_CORES  # 4096 rows per core
N = 2500
D = 256
M_TILE = 128
N_TILE = 500
M_TILES = BS // M_TILE  # 32
N_TILES = N // N_TILE  # 5

MODE = os.environ.get("BASS_SOM_MODE", "bias_add")  # "bias_add" | "matmul3"

_CACHE = {}


def _build_bass():
    import concourse.mybir as mybir
    from concourse import bacc
    from concourse.tile import TileContext

    f32 = mybir.dt.float32
    f32r = mybir.dt.float32r

    nc = bacc.Bacc("TRN2", target_bir_lowering=False, debug=False)
    xt = nc.dram_tensor("xt", [D, BS], f32r, kind="ExternalInput")
    wt = nc.dram_tensor("wt", [D, N], f32r, kind="ExternalInput")
    if MODE == "matmul3":
        xaug = nc.dram_tensor("xaug", [2, BS], f32r, kind="ExternalInput")
        waug = nc.dram_tensor("waug", [2, N], f32r, kind="ExternalInput")
    else:
        xsq_d = nc.dram_tensor("xsq", [BS], f32, kind="ExternalInput")
        wsq_d = nc.dram_tensor("wsq", [1, N], f32, kind="ExternalInput")
    out = nc.dram_tensor("out", [BS, N], f32, kind="ExternalOutput")

    n_k = 3 if MODE == "matmul3" else 2

    with TileContext(nc) as tc:
        with (
            tc.tile_pool(name="wpool", bufs=1) as wpool,
            tc.tile_pool(name="xpool", bufs=1) as xpool,
            tc.tile_pool(name="bpool", bufs=1) as bpool,
            tc.tile_pool(name="opool", bufs=4) as opool,
            tc.tile_pool(name="pp", bufs=8, space="PSUM") as pp,
        ):
            w_sb = []
            x_sb = []
            for ki in range(2):
                wk = wpool.tile([128, N], f32r, name=f"wk{ki}")
                nc.sync.dma_start(wk, wt[ki * 128 : (ki + 1) * 128, :])
                w_sb.append(wk)
            for ki in range(2):
                xk = xpool.tile([128, BS], f32r, name=f"xk{ki}")
                nc.sync.dma_start(xk, xt[ki * 128 : (ki + 1) * 128, :])
                x_sb.append(xk)

            if MODE == "matmul3":
                wk2 = wpool.tile([2, N], f32r, name="wk2")
                nc.sync.dma_start(wk2, waug[:, :])
                w_sb.append(wk2)
                xk2 = xpool.tile([2, BS], f32r, name="xk2")
                nc.sync.dma_start(xk2, xaug[:, :])
                x_sb.append(xk2)
            else:
                # ||x||^2 per row, laid out [partition=row-within-tile, m-tile]
                xsq = bpool.tile([M_TILE, M_TILES], f32)
                nc.sync.dma_start(
                    xsq, xsq_d[:].rearrange("(m p) -> p m", p=M_TILE)
                )
                # ||w||^2 replicated across all 128 partitions
                wsq_row = bpool.tile([1, N], f32)
                nc.sync.dma_start(wsq_row, wsq_d[:, :])
                wsq_bc = bpool.tile([M_TILE, N], f32)
                nc.gpsimd.partition_broadcast(wsq_bc, wsq_row[0:1, :])

            for m in range(M_TILES):
                ms = slice(m * M_TILE, (m + 1) * M_TILE)
                ot = opool.tile([M_TILE, N], f32, name="ot")
                for n in range(N_TILES):
                    ns = slice(n * N_TILE, (n + 1) * N_TILE)
                    ps = pp.tile([M_TILE, N_TILE], f32, name="ps")
                    for ki in range(n_k):
                        nc.tensor.matmul(
                            ps,
                            lhsT=x_sb[ki][:, ms],
                            rhs=w_sb[ki][:, ns],
                            start=(ki == 0),
                            stop=(ki == n_k - 1),
                        )
                    if MODE == "bias_add":
                        nc.vector.tensor_add(ps, ps, wsq_bc[:, ns])
                        nc.scalar.activation(
                            ot[:, ns],
                            ps,
                            mybir.ActivationFunctionType.Sqrt,
                            bias=xsq[:, m : m + 1],
                            scale=1.0,
                        )
                    else:
                        nc.scalar.sqrt(ot[:, ns], ps)
                nc.sync.dma_start(out[ms, :], ot)

    nc.finalize()
    return nc


def _prep_inputs(x, weights):
    x = np.ascontiguousarray(np.asarray(x, dtype=np.float32))
    w = np.ascontiguousarray(np.asarray(weights, dtype=np.float32))
    assert x.shape == (BATCH, D), x.shape
    assert w.shape == (N, D), w.shape

    xt = np.ascontiguousarray(x.T)
    wt = np.ascontiguousarray((-2.0 * w).T)
    xsq = np.einsum("bd,bd->b", x, x)
    wsq = np.einsum("nd,nd->n", w, w)

    in_maps = []
    for c in range(N_CORES):
        bs = slice(c * BS, (c + 1) * BS)
        m = {
            "xt": np.ascontiguousarray(xt[:, bs]),
            "wt": wt,
        }
        if MODE == "matmul3":
            xaug = np.empty((2, BS), dtype=np.float32)
            xaug[0] = 1.0
            xaug[1] = xsq[bs]
            waug = np.empty((2, N), dtype=np.float32)
            waug[0] = wsq
            waug[1] = 1.0
            m["xaug"] = xaug
            m["waug"] = waug
        else:
            m["xsq"] = np.ascontiguousarray(xsq[bs])
            m["wsq"] = np.ascontiguousarray(wsq[None, :])
        in_maps.append(m)
    return in_maps


def run(x, weights, trace=False, **kwargs):
    from concourse.bass_utils import run_bass_kernel_spmd

    if "nc" not in _CACHE:
        _CACHE["nc"] = _build_bass()
    nc = _CACHE["nc"]
    in_maps = _prep_inputs(x, weights)
    res = run_bass_kernel_spmd(
        nc, in_maps, core_ids=list(range(N_CORES)), trace=trace, **kwargs
    )
    out = np.concatenate([res.results[c]["out"] for c in range(N_CORES)], axis=0)
    return out, res


def kernel(x, weights):
    out, _ = run(x, weights, trace=False)
    return out


# revision 6
# speedup vs baseline: 1.1938x; 1.1938x over previous
"""Kohonen SOM distance-matrix kernel for Trainium2 (Bass/Tile).

Computes sqrt(max(||x||^2 + ||w||^2 - 2 x.w, 0)) for x [32768, 256] against a
codebook w [2500, 256] -> out [32768, 2500], data-parallel over 8 NeuronCores
(batch sharded, codebook replicated).

Per core (batch shard of 4096 rows):
  - Host preps transposed operands:
      xt [256, 4096] = x.T (float32r), wt [256, 2500] = (-2 w).T (float32r)
      xsq [128, 32] = ||x||^2 tiled [row-in-tile, m-tile], wsq [1, 2500]
  - TensorE (float32r: FP22 multiply, FP32 accumulate, full rate for N>=256)
    computes cross = -2 x.w into PSUM tiles [128, 500]; x is the stationary
    operand so output partitions = batch rows (contiguous 10KB-row stores).
  - VectorE adds broadcast ||w||^2 (PSUM -> SBUF), ScalarE computes
    sqrt(t + ||x||^2) in-place in SBUF via per-partition bias, DMA out.
  - PE warm-up matmuls run during the input-load phase to engage the HAM
    clock un-throttle (1.2 -> 2.4 GHz) before real compute starts.
"""

import os

import numpy as np

N_CORES = 8
BATCH = 32768
BS = BATCH // N_CORES  # 4096 rows per core
N = 2500
D = 256
M_TILE = 128
N_TILE = 500
M_TILES = BS // M_TILE  # 32
N_TILES = N // N_TILE  # 5
X_CHUNK = 512  # columns per x-load chunk (4 m-tiles)
X_CHUNKS = BS // X_CHUNK  # 8
WARM_MM = 16  # PE warm-up matmuls (~6.8us cold => HAM un-throttles)

_CACHE = {}


def _build_bass():
    import concourse.mybir as mybir
    from concourse import bacc
    from concourse.tile import TileContext

    f32 = mybir.dt.float32
    f32r = mybir.dt.float32r

    nc = bacc.Bacc("TRN2", target_bir_lowering=False, debug=False)
    xt = nc.dram_tensor("xt", [D, BS], f32r, kind="ExternalInput")
    wt = nc.dram_tensor("wt", [D, N], f32r, kind="ExternalInput")
    xsq_d = nc.dram_tensor("xsq", [M_TILE, M_TILES], f32, kind="ExternalInput")
    wsq_d = nc.dram_tensor("wsq", [1, N], f32, kind="ExternalInput")
    out = nc.dram_tensor("out", [BS, N], f32, kind="ExternalOutput")

    with TileContext(nc) as tc:
        with (
            tc.tile_pool(name="wpool", bufs=1) as wpool,
            tc.tile_pool(name="xpool", bufs=1) as xpool,
            tc.tile_pool(name="bpool", bufs=1) as bpool,
            tc.tile_pool(name="opool", bufs=4) as opool,
            tc.tile_pool(name="pp", bufs=7, space="PSUM") as pp,
            tc.tile_pool(name="pwarm", bufs=1, space="PSUM") as pwarm,
        ):
            # --- PE warm-up: no DMA deps, issues at t=0 while inputs load.
            warm_src = bpool.tile([M_TILE, 512], mybir.dt.bfloat16)
            nc.vector.memset(warm_src, 0.0)
            warm_ps = pwarm.tile([M_TILE, 512], f32)
            for _ in range(WARM_MM):
                nc.tensor.matmul(
                    warm_ps, lhsT=warm_src[:, :M_TILE], rhs=warm_src, start=True,
                    stop=True,
                )

            # --- input loads (ACT HWDGE queue; SP queue is reserved for
            # output stores so stores never sit behind loads in queue FIFO)
            w_sb = []
            for ki in range(2):
                wk = wpool.tile([128, N], f32r, name=f"wk{ki}")
                nc.scalar.dma_start(wk, wt[ki * 128 : (ki + 1) * 128, :])
                w_sb.append(wk)
            xsq = bpool.tile([M_TILE, M_TILES], f32)
            nc.scalar.dma_start(xsq, xsq_d[:, :])
            wsq_row = bpool.tile([1, N], f32)
            nc.scalar.dma_start(wsq_row, wsq_d[:, :])
            wsq_bc = bpool.tile([M_TILE, N], f32)
            nc.gpsimd.partition_broadcast(wsq_bc, wsq_row[0:1, :])

            x_sb = [[None] * X_CHUNKS for _ in range(2)]
            for ci in range(X_CHUNKS):
                cs = slice(ci * X_CHUNK, (ci + 1) * X_CHUNK)
                for ki in range(2):
                    xc = xpool.tile([128, X_CHUNK], f32r, name=f"x{ki}_{ci}")
                    nc.scalar.dma_start(xc, xt[ki * 128 : (ki + 1) * 128, cs])
                    x_sb[ki][ci] = xc

            # --- main loop over batch tiles
            for m in range(M_TILES):
                ms = slice(m * M_TILE, (m + 1) * M_TILE)
                mo = slice((m % 4) * M_TILE, (m % 4 + 1) * M_TILE)
                ot = opool.tile([M_TILE, N], f32, name="ot")
                for n in range(N_TILES):
                    ns = slice(n * N_TILE, (n + 1) * N_TILE)
                    ps = pp.tile([M_TILE, N_TILE], f32, name="ps")
                    for ki in range(2):
                        nc.tensor.matmul(
                            ps,
                            lhsT=x_sb[ki][m // 4][:, mo],
                            rhs=w_sb[ki][:, ns],
                            start=(ki == 0),
                            stop=(ki == 1),
                        )
                    # t = cross + ||w||^2  (PSUM -> SBUF; frees the PSUM bank
                    # with a single-producer single-consumer dep for PE)
                    nc.vector.tensor_add(ot[:, ns], ps, wsq_bc[:, ns])
                    # out = sqrt(t + ||x||^2)  in-place in SBUF
                    nc.scalar.activation(
                        ot[:, ns],
                        ot[:, ns],
                        mybir.ActivationFunctionType.Sqrt,
                        bias=xsq[:, m : m + 1],
                        scale=1.0,
                    )
                nc.sync.dma_start(out[ms, :], ot)

    nc.finalize()
    return nc


def _prep_inputs(x, weights):
    x = np.ascontiguousarray(np.asarray(x, dtype=np.float32))
    w = np.ascontiguousarray(np.asarray(weights, dtype=np.float32))
    assert x.shape == (BATCH, D), x.shape
    assert w.shape == (N, D), w.shape

    xt = np.ascontiguousarray(x.T)
    wt = np.ascontiguousarray((-2.0 * w).T)
    xsq = np.einsum("bd,bd->b", x, x)
    wsq = np.einsum("nd,nd->n", w, w)

    in_maps = []
    for c in range(N_CORES):
        bs = slice(c * BS, (c + 1) * BS)
        in_maps.append(
            {
                "xt": np.ascontiguousarray(xt[:, bs]),
                "wt": wt,
                "xsq": np.ascontiguousarray(xsq[bs].reshape(M_TILES, M_TILE).T),
                "wsq": np.ascontiguousarray(wsq[None, :]),
            }
        )
    return in_maps


def run(x, weights, trace=False, **kwargs):
    from concourse.bass_utils import run_bass_kernel_spmd

    if "nc" not in _CACHE:
        _CACHE["nc"] = _build_bass()
    nc = _CACHE["nc"]
    in_maps = _prep_inputs(x, weights)
    res = run_bass_kernel_spmd(
        nc, in_maps, core_ids=list(range(N_CORES)), trace=trace, **kwargs
    )
    out = np.concatenate([res.results[c]["out"] for c in range(N_CORES)], axis=0)
    return out, res


def kernel(x, weights):
    out, _ = run(x, weights, trace=False)
    return out
